# revision 1
# baseline (speedup 1.0000x reference)
"""Bag-attention (NRE selective attention) kernel for 8 TRN2 NeuronCores, v3.

Reference computation:
    logit_i = sum_d x[i,d] * aw[q_i,d] * rw[q_i,d]
    w       = segment_softmax(logit, seg)        (bags = contiguous ranges)
    bag[b]  = sum_{i in b} w_i * x[i]
    out     = bag @ rw.T + bias

Device computes, per sentence i, both 53-dim projections in one fused
matmul (f32 PSUM accumulation, [class, sentence] layout):
    G_i = x_i @ (aw*rw).T   -> PSUM rows 0:53
    P_i = x_i @ rw.T        -> PSUM rows 53:106
and ships both as bf16. Host finishes (all cheap, O(N*C)):
    logit_i = G[q_i, i]; e = exp(logit)
    out[b] = reduceat(e*P) / reduceat(e) + bias   (f64, np.add.reduceat)

Key TRN2 facts this design is built around (measured on these cores):
  - A dma_start's descriptors split across SDMA engines in equal contiguous
    groups, n_groups = largest divisor of the partition dim <= 16.
    115 partitions -> 5 engines (~110 GB/s); 128 -> 16 engines (~310 GB/s,
    HBM-bound). So x ships as a [128, 5, ns] main block + [50, ns] tail,
    and the output uses 108 rows (12 engines).
  - Matmul output must fit one PSUM bank (512 f32) -> 512-col slices.
  - Matmul cost is pure streaming (no fixed cost) but the PE clock ramps:
    0.83 ns/cycle until ~3us of continuous execution, 0.42 after. Fewer
    sync points -> longer PE bursts -> full clock.

Sharding: 16384 contiguous sentences per core; weights replicated; the
ragged segment-sum runs on the host so bags straddling core boundaries
need no special handling. No collectives.
"""

import sys

_REPO = "/opt/trn_rl_repo"
if _REPO not in sys.path:
    sys.path.insert(0, _REPO)

import numpy as np
import ml_dtypes

N_SENT = 131072
REL_DIM = 690
C = 53
WC = 2 * C             # fused weight cols: [G 0:53 | P 53:106]
NCORES = 8
NS = N_SENT // NCORES  # sentences per core
PMAIN = 128            # main-chunk partition dim (16 SDMA engines)
NCH = 3                # bf16 main chunks: 3*128 = 384
NCH8 = 2               # fp8 (e4m3) chunks: 2*128 = 256 dims shipped as fp8
TAIL = 50              # bf16 tail dims (384:434); fp8 dims are 434:690
BLK = 2048             # sentences per compute block
SUB = 512              # matmul moving-col slice (one PSUM bank = 512 f32)
DBLK = 2048            # sentences per DMA block (= BLK; all 8 slots resident)
OROWS = 108            # output DMA rows (106 + 2 pad; 108 = 12*9 -> 12 engines)

_NC_CACHE = {}


def _build(ns):
    import concourse.bass as bass
    from concourse import mybir

    f32 = mybir.dt.float32
    bf16 = mybir.dt.bfloat16

    nblk = ns // BLK
    ndblk = ns // DBLK
    bpd = DBLK // BLK
    nslot = min(8, ndblk)

    f8 = mybir.dt.float8e4

    nc = bass.Bass()
    xm = nc.declare_dram_parameter("xm", [PMAIN * NCH, ns], bf16, isOutput=False)
    xt50 = nc.declare_dram_parameter("xt50", [TAIL, ns], bf16, isOutput=False)
    x8 = nc.declare_dram_parameter("x8", [PMAIN * NCH8, ns], f8, isOutput=False)
    wmm = nc.declare_dram_parameter("wmm", [PMAIN, NCH * WC], bf16, isOutput=False)
    wmt = nc.declare_dram_parameter("wmt", [TAIL, WC], bf16, isOutput=False)
    wm8 = nc.declare_dram_parameter("wm8", [PMAIN, NCH8 * WC], bf16, isOutput=False)
    out = nc.declare_dram_parameter("out", [OROWS, ns], bf16, isOutput=True)

    xm_r = xm[:].rearrange("(c p) n -> p c n", p=PMAIN)
    x8_r = x8[:].rearrange("(c p) n -> p c n", p=PMAIN)

    from contextlib import ExitStack
    with ExitStack() as stk:
        xbuf = stk.enter_context(nc.sbuf_tensor("xbuf", [PMAIN, nslot, NCH, DBLK], bf16))
        xtail = stk.enter_context(nc.sbuf_tensor("xtail", [TAIL, nslot, DBLK], bf16))
        x8buf = stk.enter_context(nc.sbuf_tensor("x8buf", [PMAIN, nslot, NCH8, DBLK], f8))
        wm_sb = stk.enter_context(nc.sbuf_tensor("wm_sb", [PMAIN, NCH, WC], bf16))
        wmt_sb = stk.enter_context(nc.sbuf_tensor("wmt_sb", [TAIL, WC], bf16))
        wm8_sb = stk.enter_context(nc.sbuf_tensor("wm8_sb", [PMAIN, NCH8, WC], bf16))
        out_sb = stk.enter_context(nc.sbuf_tensor("out_sb", [OROWS, 2, BLK], bf16))
        psb = [stk.enter_context(nc.psum_tensor(f"ps{i}", [WC, BLK], f32))
               for i in range(2)]

        s_xm = [stk.enter_context(nc.semaphore(f"s_xm{i}")) for i in range(ndblk)]
        s_xt = [stk.enter_context(nc.semaphore(f"s_xt{i}")) for i in range(ndblk)]
        s_wm = stk.enter_context(nc.semaphore("s_wm"))
        s_mm = stk.enter_context(nc.semaphore("s_mm"))
        s_cp = stk.enter_context(nc.semaphore("s_cp"))
        s_od = stk.enter_context(nc.semaphore("s_od"))
        block = stk.enter_context(nc.Block())

        @block.sync
        def _(sync):
            for db in range(ndblk):
                slot = db % nslot
                sync.dma_start(
                    out=xbuf[:, slot, :, :],
                    in_=xm_r[:, :, db * DBLK:(db + 1) * DBLK],
                ).then_inc(s_xm[db], 16)

        @block.gpsimd
        def _(gp):
            for db in range(ndblk):
                slot = db % nslot
                # just-in-time: don't steal stream bandwidth from earlier
                # dblocks (tail db is only needed alongside main db)
                if db >= 1:
                    gp.wait_ge(s_xm[db - 1], 16)
                gp.dma_start(
                    out=xtail[:, slot, :],
                    in_=xt50[:, db * DBLK:(db + 1) * DBLK],
                ).then_inc(s_xt[db], 16)
                gp.dma_start(
                    out=x8buf[:, slot, :, :],
                    in_=x8_r[:, :, db * DBLK:(db + 1) * DBLK],
                ).then_inc(s_xt[db], 16)

        @block.tensor
        def _(pe):
            pe.wait_ge(s_wm, 48)
            for b in range(nblk):
                db = b // bpd
                slot = db % nslot
                i = b % 2
                off = (b % bpd) * BLK
                if b % bpd == 0:
                    pe.wait_ge(s_xm[db], 16)
                    pe.wait_ge(s_xt[db], 32)
                if b >= 2:
                    pe.wait_ge(s_cp, b - 1)  # copy(b-2) freed ps[i]
                for c in range(NCH):
                    for sub in range(BLK // SUB):
                        s0, s1 = sub * SUB, (sub + 1) * SUB
                        nc.tensor.matmul(
                            psb[i][:, s0:s1],
                            wm_sb[:, c, :],
                            xbuf[:, slot, c, off + s0:off + s1],
                            start=(c == 0), stop=False,
                        )
                for c in range(NCH8):
                    for sub in range(BLK // SUB):
                        s0, s1 = sub * SUB, (sub + 1) * SUB
                        nc.tensor.matmul(
                            psb[i][:, s0:s1],
                            wm8_sb[:, c, :],
                            x8buf[:, slot, c, off + s0:off + s1],
                            start=False, stop=False,
                        )
                for sub in range(BLK // SUB):
                    s0, s1 = sub * SUB, (sub + 1) * SUB
                    mmt = nc.tensor.matmul(
                        psb[i][:, s0:s1], wmt_sb[:],
                        xtail[:, slot, off + s0:off + s1],
                        start=False, stop=True,
                    )
                mmt.then_inc(s_mm, 1)

        @block.vector
        def _(dve):
            nc.vector.memset(out_sb[:, :, :], 0.0)
            for b in range(nblk):
                i = b % 2
                dve.wait_ge(s_mm, b + 1)
                if b >= 2:
                    dve.wait_ge(s_od, 16 * (b - 1))  # out-DMA(b-2) freed out_sb[i]
                nc.vector.tensor_copy(
                    out_sb[0:WC, i, :], psb[i][:, :]
                ).then_inc(s_cp, 1)

        @block.scalar
        def _(act):
            nc.scalar.dma_start(out=wm_sb[:], in_=wmm[:]).then_inc(s_wm, 16)
            nc.scalar.dma_start(out=wmt_sb[:], in_=wmt[:]).then_inc(s_wm, 16)
            nc.scalar.dma_start(out=wm8_sb[:], in_=wm8[:]).then_inc(s_wm, 16)
            for b in range(nblk):
                i = b % 2
                act.wait_ge(s_cp, b + 1)
                nc.scalar.dma_start(
                    out=out[:, b * BLK:(b + 1) * BLK],
                    in_=out_sb[:, i, :],
                ).then_inc(s_od, 16)

    return nc


def _get_nc(ns=NS):
    if ns not in _NC_CACHE:
        _NC_CACHE[ns] = _build(ns)
    return _NC_CACHE[ns]


def _prepare(x, relation_weight, attention_weight):
    bf16 = ml_dtypes.bfloat16
    e4m3 = ml_dtypes.float8_e4m3
    x = np.asarray(x, dtype=np.float32)
    rw = np.asarray(relation_weight, dtype=np.float32)
    aw = np.asarray(attention_weight, dtype=np.float32)

    n = x.shape[0]
    ns = n // NCORES
    NB = PMAIN * NCH          # 384 bf16 dims
    NT = NB + TAIL            # 434; dims 434:690 ship as e4m3

    # fused weights: cols 0:53 = (aw*rw).T (G path), cols 53:106 = rw.T (P)
    wmat = np.concatenate([(aw * rw).T, rw.T], axis=1).astype(bf16)

    def pack(wslice, nch):
        return np.ascontiguousarray(
            wslice.reshape(nch, PMAIN, WC).transpose(1, 0, 2)
            .reshape(PMAIN, nch * WC))

    wmm_p = pack(wmat[0:NB], NCH)
    wmt_p = np.ascontiguousarray(wmat[NB:NT])
    wm8_p = pack(wmat[NT:], NCH8)

    xb = x.astype(bf16)
    x8b = x[:, NT:].astype(e4m3)
    in_maps = []
    for m in range(NCORES):
        sl = slice(m * ns, (m + 1) * ns)
        xt = np.ascontiguousarray(xb[sl, 0:NT].T)
        in_maps.append({
            "xm": xt[0:NB],
            "xt50": np.ascontiguousarray(xt[NB:NT]),
            "x8": np.ascontiguousarray(x8b[sl].T),
            "wmm": wmm_p,
            "wmt": wmt_p,
            "wm8": wm8_p,
        })
    return in_maps


def _combine(outs, attention_query, scope, bias):
    """outs: [NCORES, OROWS, ns] bf16 with rows 0:53 = G, 53:106 = P.
    Host: logit gather, exp, e*P, segment sums over contiguous bags."""
    ns = NS
    GP = np.asarray(outs, dtype=np.float32)            # [8, 108, ns]
    G = GP[:, 0:C, :].transpose(0, 2, 1).reshape(N_SENT, C)
    P = GP[:, C:WC, :].transpose(0, 2, 1).reshape(N_SENT, C).astype(np.float64)
    q = np.asarray(attention_query).astype(np.int64)
    logit = G[np.arange(N_SENT), q].astype(np.float64)
    e = np.exp(logit)
    epe = P * e[:, None]
    scope = np.asarray(scope).astype(np.int64)
    sums = np.add.reduceat(epe, scope[:-1], axis=0)
    esum = np.add.reduceat(e, scope[:-1])
    logits = sums / esum[:, None] + np.asarray(bias, np.float64)[None, :]
    return logits.astype(np.float32)


def _run(inputs, trace=False, **kw):
    from concourse.bass_utils import run_bass_kernel_spmd

    nc = _get_nc(NS)
    in_maps = _prepare(
        inputs["x"], inputs["relation_weight"], inputs["attention_weight"],
    )
    res = run_bass_kernel_spmd(nc, in_maps, core_ids=list(range(NCORES)),
                               trace=trace, **kw)
    outs = np.stack([np.asarray(r["out"]) for r in res.results])
    logits = _combine(outs, inputs["attention_query"], inputs["scope"],
                      np.asarray(inputs["bias"], np.float32))
    return logits, res


def kernel(x, relation_weight, attention_weight, bias, attention_query, scope):
    logits, _ = _run(dict(x=x, relation_weight=relation_weight,
                          attention_weight=attention_weight, bias=bias,
                          attention_query=attention_query, scope=scope))
    return logits

